# revision 18
# baseline (speedup 1.0000x reference)
"""2-layer GAT (PyG GATConv semantics) on 8 Trainium2 NeuronCores.

Sharding (graph partitioning per the hint): nodes split 8 ways (12500/core);
each core owns the edges whose destination falls in its node range
(dst-sorted, padded to 128-edge tiles per 128-node block; identical tile
structure across cores so one SPMD program serves all).

Measured: rel err 9.2e-5 vs the fp32 reference on the 8-core axon TRN2;
cost-model timeline 3.839ms/core (prev revisions 4.338 -> 4.035 -> 3.858ms;
graded baseline 5.283ms). Schedule: 347us startup (phase A ~60us,
latency-bound across engines + table1 AllGather 286us = 27.3MB at the
model's size-interpolated ~101GB/s + 15us, verified exact), then a dense
gather stream (2x1662 tiles x ~1.04us of serial Pool-engine SWDGE
descriptor-gen each; 994ns fixed + 0.34ns/desc), one 38us gap (last
table2-chunk AllGather tail), 9us drain. The gather count is the floor on
this toolchain: every amortized-gather form is hardware-verified broken
([128,k] offsets generate only partition-0 descriptors; [128,1] offsets
with a [128,2,D] out - the consecutive-row-pair fetch that would enable a
host-side edge-pairing design at ~-50% gathers - returns wrong data on all
128 partitions; dma_gather raises NRT_EXEC_UNIT_UNRECOVERABLE;
remote_dma/_broadcast are the same custom-ISA family and SBUF->SBUF only). The table2 AllGather is chunk-major +
split in N_CH pieces fired from the L1 block closes (two blocks late so
their sem waits never stall the Pool sequencer), hiding ~195us under the
L1 gather stream; table1's cannot overlap anything (the first gather needs
the full table, and chunking it loses outright to the collective model's
size-dependent bandwidth curve).

vs the previous revision (~5.3ms graded):
  - Self-loops are no longer materialized as gathered edge tiles: each
    128-node block adds its self-loop contribution with one identity-matmul
    from the SBUF-resident own-shard [h | a_src] + a_dst tables
    (-2*98 SWDGE gathers, the dominant cost at ~1us each).
  - Tables slimmed to [h | a_src] (a_dst never gathered): 136/41 cols
    instead of 144/42 -> smaller AllGathers + gather rows.
  - Layer-2 shard compute (h1 -> h2) fused into the layer-1 block close,
    removing the serial phase-C segment.
  - One-hot built per tile with tensor_scalar(is_equal, scalar1=dl column):
    all-SBUF f16 -> 4x DVE mode. ohT PSUM->SBUF copies batched 4 tiles per
    ACT instruction.

Per layer:
  1. Each core computes its shard of the node table [h | a_src] (+a_dst
     kept SBUF-only) and the shards are AllGathered into a replicated fp16
     table (the "halo exchange").
  2. Per 128-edge tile: one indirect-DMA gather of the 128 src rows
     (the only gather primitive that works on this toolchain; batched
     offsets and dma_gather corrupt data / crash the device - verified),
     a one-hot (iota == dst_local) on DVE, PE-transpose of the one-hot +
     a tiny matmul to broadcast per-edge a_dst, exp(max(s, 0.2 s)) without
     max-subtraction (values are small, safe), then a one-hot scatter
     matmul accumulating [alpha*h_src | alpha] per block in PSUM.
  3. Block close: numer/denom (+1e-16), bias, elu (layer 1, then fused
     h2-shard compute) or log_softmax (layer 2, output shard).

Toolchain workarounds: this walrus accepts max ONE sem wait/instruction
(_split_multi_waits hoists extras onto NOPs); gpsimd custom ISA ops
(dma_gather/...) crash the device; indirect_dma_start only works with
[128,1] offset lists (one offset per partition).
"""
import sys
sys.path.insert(0, "/opt/trn_rl_repo")
import numpy as np

import concourse.bass as bass
import concourse.tile as tile
from concourse import mybir
P = 128
N_CORES = 8
N_CH = 20
NEG_SLOPE = 0.2
G_BUFS = 4
SCRATCH = 16384
# PSUM banks: psA(2) + ohtps + adp + acc must stay <= 8
TUNE = dict(wp=3, gp=4, acc=2, adp=2, ohtps=2, oht_on_dve=False)
F32 = mybir.dt.float32
F16 = mybir.dt.float16
I32 = mybir.dt.int32


def _split_multi_waits(nc):
    """This walrus build accepts at most one sem wait per instruction; hoist
    extras onto preceding same-engine NOPs (sequencers run in order)."""
    ctr = 0
    for bb in nc.main_func.blocks:
        new = []
        changed = False
        for ins in bb.instructions:
            si = ins.sync_info
            waits = list(si.on_wait) if si is not None and si.on_wait else []
            if len(waits) > 1:
                changed = True
                for w in waits[:-1]:
                    ctr += 1
                    new.append(mybir.InstNoOp(
                        name=f"wsplit_{ctr}", ins=[], outs=[], engine=ins.engine,
                        sync_info=mybir.SyncInfo(on_wait=[w], on_update=[])))
                si.on_wait = waits[-1:]
            new.append(ins)
        if changed:
            bb.instructions = new


def _host_prep(edge_index, n_nodes):
    """Integer-only preprocessing: shard by dst, sort, pad to 128-edge tiles
    per 128-node block; identical tile structure across cores (SPMD).
    Self-loops are NOT added here - the kernel applies them per block via an
    identity matmul (reference appends one loop per node)."""
    npc = n_nodes // N_CORES               # nodes per core
    nb = (npc + P - 1) // P                # blocks per core
    npc_pad = nb * P
    src = edge_index[0].astype(np.int64)
    dst = edge_index[1].astype(np.int64)

    per_core = []
    counts_all = np.zeros((N_CORES, nb), np.int64)
    for c in range(N_CORES):
        sel = (dst // npc) == c
        ls = src[sel]
        ld = dst[sel] - c * npc
        order = np.argsort(ld, kind="stable")
        ls, ld = ls[order], ld[order]
        blk = ld // P
        counts_all[c] = np.bincount(blk, minlength=nb)
        per_core.append((ls, ld, blk))

    tiles_per_block = -(-counts_all.max(axis=0) // P)  # ceil; 0 allowed
    cum_tiles = np.concatenate([[0], np.cumsum(tiles_per_block)])
    nt = int(cum_tiles[-1])

    # table2 is laid out chunk-major so its AllGather can be chunked and
    # overlapped with the L1 edge phase (chunk i gatherable as soon as the
    # L1 closes for its blocks are done). N_CH chunks of blocks; table2 row
    # for (core, local r): ch = chunk(r//P);
    #   trow2 = 8*c0r[ch] + core*chrows[ch] + (r - c0r[ch])
    ch_blocks = [nb // N_CH + (1 if i < nb % N_CH else 0) for i in range(N_CH)]
    c0b = np.concatenate([[0], np.cumsum(ch_blocks)])
    c0r = c0b * P
    chrows = np.diff(c0r)
    chunk_of_block = np.searchsorted(c0b, np.arange(nb), side="right") - 1

    src_idx_all, src_idx2_all, dst_loc_all = [], [], []
    for c in range(N_CORES):
        ls, ld, blk = per_core[c]
        starts = np.concatenate([[0], np.cumsum(counts_all[c])])
        rank = np.arange(len(ls)) - starts[blk]
        pos = P * cum_tiles[blk] + rank
        flat_src = np.zeros(nt * P, np.int32)        # pad: gather row 0
        flat_src2 = np.zeros(nt * P, np.int32)
        flat_dl = np.full(nt * P, -1.0, np.float32)  # pad: no one-hot match
        # remap global node id -> padded table row (core*npc_pad + local)
        ls_core = ls // npc
        lr = ls - ls_core * npc
        flat_src[pos] = (ls_core * npc_pad + lr).astype(np.int32)
        sch = chunk_of_block[lr // P]
        flat_src2[pos] = (N_CORES * c0r[sch] + ls_core * chrows[sch]
                          + (lr - c0r[sch])).astype(np.int32)
        flat_dl[pos] = (ld - P * blk).astype(np.float32)
        src_idx_all.append(flat_src.reshape(nt, P).T.copy())   # [128, nt]
        src_idx2_all.append(flat_src2.reshape(nt, P).T.copy())
        dst_loc_all.append(flat_dl.reshape(nt, P).T.copy())    # [128, nt]

    bake = dict(npc=npc, nb=nb, npc_pad=npc_pad, nt=nt,
                tiles_per_block=[int(t) for t in tiles_per_block],
                cum_tiles=[int(t) for t in cum_tiles],
                ch_c0r=[int(x) for x in c0r],
                ch_end_blocks=[int(x) - 1 for x in c0b[1:]])
    return bake, src_idx_all, src_idx2_all, dst_loc_all


def _build_program(bake, nfeat, nhid, heads, nclass):
    """Emit the SPMD bass program (same for all cores)."""
    npc, nb, npc_pad, nt = bake["npc"], bake["nb"], bake["npc_pad"], bake["nt"]
    cumt = bake["cum_tiles"]
    hh = heads * nhid            # 128
    d1 = hh + heads              # 136: [h | a_src]   (gathered)
    d1e = d1 + heads             # 144: + a_dst       (compute width)
    d2 = nclass + 1              # 41:  [h2 | a_src2]
    d2e = d2 + 1                 # 42:  + a_dst2
    ntab = N_CORES * npc_pad

    nc = bass.Bass(dynamic_dma_scratch_size=SCRATCH)
    xT = nc.dram_tensor("xT", [nfeat, npc], F32, kind="ExternalInput")
    W1 = nc.dram_tensor("W1", [nfeat, hh], F32, kind="ExternalInput")
    W2 = nc.dram_tensor("W2", [hh, nclass], F32, kind="ExternalInput")
    asrc1 = nc.dram_tensor("asrc1", [P, hh], F32, kind="ExternalInput")
    adst1 = nc.dram_tensor("adst1", [P, hh], F32, kind="ExternalInput")
    asrc2 = nc.dram_tensor("asrc2", [P, nclass], F32, kind="ExternalInput")
    adst2 = nc.dram_tensor("adst2", [P, nclass], F32, kind="ExternalInput")
    b1r = nc.dram_tensor("b1r", [P, hh], F32, kind="ExternalInput")
    b2r = nc.dram_tensor("b2r", [P, nclass], F32, kind="ExternalInput")
    srcidx = nc.dram_tensor("srcidx", [P, nt], I32, kind="ExternalInput")
    srcidx2 = nc.dram_tensor("srcidx2", [P, nt], I32, kind="ExternalInput")
    iota_in = nc.dram_tensor("iota128", [P, P], F32, kind="ExternalInput")
    ident_in = nc.dram_tensor("ident128", [P, P], F32, kind="ExternalInput")
    dstloc = nc.dram_tensor("dstloc", [P, nt], F32, kind="ExternalInput")
    out = nc.dram_tensor("out", [npc, nclass], F32, kind="ExternalOutput")

    shard1 = nc.dram_tensor("shard1", [npc_pad, d1], F16)
    table1 = nc.dram_tensor("table1", [ntab, d1], F16, addr_space="Shared")
    shard2 = nc.dram_tensor("shard2", [npc_pad, d2], F16)
    table2 = nc.dram_tensor("table2", [ntab, d2], F16, addr_space="Shared")

    AF = mybir.ActivationFunctionType
    OP = mybir.AluOpType

    with tile.TileContext(nc) as tc:
        with tc.tile_pool(name="persist", bufs=1) as pp, \
             tc.tile_pool(name="work", bufs=TUNE["wp"]) as wp, \
             tc.tile_pool(name="gpool", bufs=TUNE["gp"]) as gp, \
             tc.tile_pool(name="psA", bufs=2, space="PSUM") as psA, \
             tc.tile_pool(name="psT", bufs=2, space="PSUM") as psT, \
             tc.tile_pool(name="psB", bufs=TUNE["acc"], space="PSUM") as psB:

            # ---- constants (host-supplied)
            iota_f = pp.tile([P, P], F32)
            ident = pp.tile([P, P], F32)
            nc.sync.dma_start(iota_f[:], iota_in[:])
            nc.sync.dma_start(ident[:], ident_in[:])
            iota16 = pp.tile([P, P], F16)
            ident16 = pp.tile([P, P], F16)
            nc.vector.tensor_copy(iota16[:], iota_f[:])
            nc.vector.tensor_copy(ident16[:], ident[:])

            dl = pp.tile([P, nt], F32)
            si_t = pp.tile([P, nt], I32)
            si2_t = pp.tile([P, nt], I32)
            nc.sync.dma_start(dl[:], dstloc[:])
            nc.sync.dma_start(si_t[:], srcidx[:])
            nc.sync.dma_start(si2_t[:], srcidx2[:])

            b1_t = pp.tile([P, hh], F32)
            b2_t = pp.tile([P, nclass], F32)
            nc.sync.dma_start(b1_t[:], b1r[:])
            nc.sync.dma_start(b2_t[:], b2r[:])

            # own-shard tables for the self-loop path (SBUF-resident)
            hs1 = pp.tile([P, nb, d1], F16)       # [h | a_src] per block
            ad1own = pp.tile([P, nb, heads], F16)  # a_dst per block
            hs2 = pp.tile([P, nb, d2], F16)
            ad2own = pp.tile([P, nb, 1], F16)

            # ---- phase A: W1_ext, h_ext shard, allgather table1
            w1_t = wp.tile([nfeat, hh], F32, tag="w1")
            nc.sync.dma_start(w1_t[:], W1[:])
            as1 = wp.tile([P, hh], F32, tag="as1")
            ad1 = wp.tile([P, hh], F32, tag="ad1")
            nc.sync.dma_start(as1[:], asrc1[:])
            nc.sync.dma_start(ad1[:], adst1[:])
            w1e = pp.tile([nfeat, d1e], F32)
            nc.scalar.copy(w1e[:, 0:hh], w1_t[:])
            tmp = wp.tile([P, hh], F32, tag="tmpw")
            nc.vector.tensor_tensor(out=tmp[:], in0=w1_t[:], in1=as1[:], op=OP.mult)
            nc.vector.tensor_reduce(
                out=w1e[:, hh:d1],
                in_=tmp[:].rearrange("p (h c) -> p h c", h=heads),
                axis=mybir.AxisListType.X, op=OP.add)
            nc.vector.tensor_tensor(out=tmp[:], in0=w1_t[:], in1=ad1[:], op=OP.mult)
            nc.vector.tensor_reduce(
                out=w1e[:, d1:d1e],
                in_=tmp[:].rearrange("p (h c) -> p h c", h=heads),
                axis=mybir.AxisListType.X, op=OP.add)

            # 8 blocks per HWDGE load/store (the 625ns fixed overhead per
            # DMA instruction paces phase A otherwise)
            for b0 in range(0, nb, 8):
                kk = min(8, nb - b0)
                cols = min(8 * P, npc - b0 * P)
                xTb = wp.tile([nfeat, 8 * P], F32, tag="xTb")
                nc.sync.dma_start(xTb[:, 0:cols], xT[:, b0 * P:b0 * P + cols])
                stg4 = wp.tile([P, 8, d1e], F16, tag="stg1")
                if b0 + 8 >= nb:
                    nc.vector.memset(stg4[:], 0.0)
                for j in range(kk):
                    b = b0 + j
                    cnt = min(P, npc - b * P)
                    ps = psA.tile([P, d1e], F32, tag="ps_a")
                    nc.tensor.matmul(out=ps[:cnt, :],
                                     lhsT=xTb[:, j * P:j * P + cnt],
                                     rhs=w1e[:], start=True, stop=True)
                    nc.scalar.copy(stg4[:cnt, j, :], ps[:cnt, :])
                    nc.vector.tensor_copy(hs1[:, b, :], stg4[:, j, 0:d1])
                    nc.vector.tensor_copy(ad1own[:, b, :], stg4[:, j, d1:d1e])
                nc.sync.dma_start(
                    shard1[b0 * P:(b0 + kk) * P, :].rearrange(
                        "(j p) c -> p j c", p=P),
                    stg4[:, 0:kk, 0:d1])
            nc.gpsimd.collective_compute(
                "AllGather", OP.bypass,
                replica_groups=[list(range(N_CORES))],
                ins=[shard1[:]], outs=[table1[:]])

            # ---- edge phase helper
            def edge_layer(tab, si, dcols, hcols, nheads, hdim, adown,
                           hsown, close_fn, post_close=None):
                """dcols: gathered row width; hcols: feature cols; per block:
                identity self-loop matmul, then per-tile gather / one-hot /
                a_dst / alpha / scatter."""
                rhsw = hcols + nheads      # [v | alpha]

                def make_self(b):
                    # self-loop contribution (src == dst, local): DVE/ACT
                    # chain emitted one block ahead so the PE matmul that
                    # consumes rhs_s never stalls on it
                    s8 = wp.tile([P, nheads], F32, tag="s8")
                    nc.vector.tensor_tensor(
                        out=s8[:], in0=hsown[:, b, hcols:hcols + nheads],
                        in1=adown[:, b, :], op=OP.add)
                    nc.vector.scalar_tensor_tensor(
                        out=s8[:], in0=s8[:], scalar=NEG_SLOPE, in1=s8[:],
                        op0=OP.mult, op1=OP.max)
                    e8 = wp.tile([P, nheads], F16, tag="e8")
                    nc.scalar.activation(e8[:], s8[:], AF.Exp)
                    rhs_s = wp.tile([P, rhsw], F16, tag="rhs_s")
                    nc.vector.tensor_tensor(
                        out=rhs_s[:, 0:hcols].rearrange(
                            "p (h c) -> p h c", h=nheads),
                        in0=hsown[:, b, 0:hcols].rearrange(
                            "p (h c) -> p h c", h=nheads),
                        in1=e8[:].unsqueeze(2).to_broadcast(
                            [P, nheads, hdim]),
                        op=OP.mult)
                    nc.vector.tensor_copy(rhs_s[:, hcols:rhsw], e8[:])
                    return rhs_s

                rhs_self = make_self(0)
                pending = None            # (b, acc) awaiting deferred close
                for b in range(nb):
                    acc = psB.tile([P, rhsw], F32, tag="acc")
                    t0, t1 = cumt[b], cumt[b + 1]
                    nc.tensor.matmul(out=acc[:], lhsT=ident16[:],
                                     rhs=rhs_self[:],
                                     start=True, stop=(t0 == t1),
                                     skip_group_check=True)
                    if b + 1 < nb:
                        rhs_self = make_self(b + 1)
                    # -- gathered tiles
                    t = t0
                    while t < t1:
                        nb_t = min(4, t1 - t)
                        G = gp.tile([P, 4, dcols], F16, tag="G")
                        for k in range(nb_t):
                            nc.gpsimd.indirect_dma_start(
                                out=G[:, k, :], out_offset=None, in_=tab[:],
                                in_offset=bass.IndirectOffsetOnAxis(
                                    ap=si[:, t + k:t + k + 1], axis=0))
                        oh = wp.tile([P, 4, P], F16, tag="oh")
                        for k in range(nb_t):
                            nc.vector.tensor_scalar(
                                out=oh[:, k, :], in0=iota16[:],
                                scalar1=dl[:, t + k:t + k + 1], scalar2=None,
                                op0=OP.is_equal)
                        ohT_ps = psT.tile([P, 4, P], F16, tag="ohT_ps", bufs=TUNE["ohtps"])
                        for k in range(nb_t):
                            nc.tensor.transpose(out=ohT_ps[:, k, :],
                                                in_=oh[:, k, :],
                                                identity=ident16[:])
                        ohT = wp.tile([P, 4, P], F16, tag="ohT")
                        if TUNE["oht_on_dve"]:
                            nc.vector.tensor_copy(ohT[:, 0:nb_t, :],
                                                  ohT_ps[:, 0:nb_t, :])
                        else:
                            nc.scalar.copy(ohT[:, 0:nb_t, :], ohT_ps[:, 0:nb_t, :])
                        adp = psT.tile([P, 4 * nheads], F32, tag="adp", bufs=TUNE["adp"])
                        for k in range(nb_t):
                            nc.tensor.matmul(
                                out=adp[:, k * nheads:(k + 1) * nheads],
                                lhsT=ohT[:, k, :],
                                rhs=adown[:, b, :],
                                start=True, stop=True)
                        rhs = wp.tile([P, 4, rhsw], F16, tag="rhs")
                        s_t = wp.tile([P, 4 * nheads], F32, tag="s")
                        nc.vector.tensor_tensor(
                            out=s_t[:, 0:nb_t * nheads].rearrange(
                                "p (t h) -> p t h", h=nheads),
                            in0=G[:, 0:nb_t, hcols:hcols + nheads],
                            in1=adp[:, 0:nb_t * nheads].rearrange(
                                "p (t h) -> p t h", h=nheads),
                            op=OP.add)
                        nc.vector.scalar_tensor_tensor(
                            out=s_t[:, 0:nb_t * nheads],
                            in0=s_t[:, 0:nb_t * nheads], scalar=NEG_SLOPE,
                            in1=s_t[:, 0:nb_t * nheads],
                            op0=OP.mult, op1=OP.max)
                        nc.scalar.activation(
                            rhs[:, 0:nb_t, hcols:rhsw],
                            s_t[:, 0:nb_t * nheads].rearrange(
                                "p (t h) -> p t h", h=nheads),
                            AF.Exp)
                        nc.vector.tensor_tensor(
                            out=rhs[:, 0:nb_t, 0:hcols],
                            in0=G[:, 0:nb_t, 0:hcols],
                            in1=rhs[:, 0:nb_t, hcols:rhsw].unsqueeze(
                                3).to_broadcast([P, nb_t, nheads, hdim]),
                            op=OP.mult)
                        for k in range(nb_t):
                            nc.tensor.matmul(
                                out=acc[:], lhsT=oh[:, k, :], rhs=rhs[:, k, :],
                                start=False, stop=(t + k == t1 - 1),
                                skip_group_check=True)
                        t += nb_t
                    if pending is not None:
                        close_fn(*pending)
                        if post_close is not None:
                            post_close(pending[0])
                    pending = (b, acc)
                if pending is not None:
                    close_fn(*pending)
                    if post_close is not None:
                        post_close(pending[0])

            # ---- L1 close: normalize + bias + elu -> h1 block, then fused
            #      L2 shard compute (h2 = h1 @ W2_ext) + shard2 store
            def close1(b, acc):
                n0 = b * P
                cnt = min(P, npc - n0)
                d8 = wp.tile([P, heads], F32, tag="d8")
                nc.vector.tensor_scalar(out=d8[:], in0=acc[:, hh:hh + heads],
                                        scalar1=1e-16, scalar2=None, op0=OP.add)
                r8 = wp.tile([P, heads], F32, tag="r8")
                nc.vector.reciprocal(r8[:], d8[:])
                tt = wp.tile([P, hh], F32, tag="tt")
                nc.vector.tensor_tensor(
                    out=tt[:].rearrange("p (h c) -> p h c", h=heads),
                    in0=acc[:, 0:hh].rearrange("p (h c) -> p h c", h=heads),
                    in1=r8[:].unsqueeze(2).to_broadcast([P, heads, nhid]),
                    op=OP.mult)
                nc.vector.tensor_tensor(out=tt[:], in0=tt[:], in1=b1_t[:],
                                        op=OP.add)
                pos = wp.tile([P, hh], F32, tag="pos")
                neg = wp.tile([P, hh], F32, tag="neg")
                nc.vector.tensor_scalar(out=pos[:], in0=tt[:], scalar1=0.0,
                                        scalar2=None, op0=OP.max)
                nc.vector.tensor_scalar(out=neg[:], in0=tt[:], scalar1=0.0,
                                        scalar2=None, op0=OP.min)
                nc.scalar.activation(neg[:], neg[:], AF.Exp)
                h1b = wp.tile([P, hh], F32, tag="h1b")
                nc.vector.scalar_tensor_tensor(
                    out=h1b[:], in0=pos[:], scalar=-1.0, in1=neg[:],
                    op0=OP.add, op1=OP.add)
                # fused phase C for this block
                tps = psA.tile([P, P], F32, tag="ps_a")
                nc.tensor.transpose(out=tps[:], in_=h1b[:], identity=ident[:])
                h1T = wp.tile([P, P], F32, tag="h1T")
                nc.scalar.copy(h1T[:], tps[:])
                ps2 = psA.tile([P, d2e], F32, tag="ps_a")
                nc.tensor.matmul(out=ps2[:cnt, :], lhsT=h1T[:, 0:cnt],
                                 rhs=w2e[:], start=True, stop=True)
                stg2 = wp.tile([P, d2e], F16, tag="stg2")
                if cnt < P:
                    nc.vector.memset(stg2[:], 0.0)
                nc.scalar.copy(stg2[:cnt, :], ps2[:cnt, :])
                nc.vector.tensor_copy(hs2[:, b, :], stg2[:, 0:d2])
                nc.vector.tensor_copy(ad2own[:, b, :], stg2[:, d2:d2e])
                nc.sync.dma_start(shard2[n0:n0 + P, :], stg2[:, 0:d2])

            # W2_ext built up front (needed by fused close1)
            w2_t = wp.tile([hh, nclass], F32, tag="w2")
            nc.sync.dma_start(w2_t[:], W2[:])
            as2 = wp.tile([P, nclass], F32, tag="as2")
            ad2 = wp.tile([P, nclass], F32, tag="ad2")
            nc.sync.dma_start(as2[:], asrc2[:])
            nc.sync.dma_start(ad2[:], adst2[:])
            w2e = pp.tile([hh, d2e], F32)
            nc.scalar.copy(w2e[:, 0:nclass], w2_t[:])
            tmp2 = wp.tile([P, nclass], F32, tag="tmp2")
            nc.vector.tensor_tensor(out=tmp2[:], in0=w2_t[:], in1=as2[:], op=OP.mult)
            nc.vector.tensor_reduce(out=w2e[:, nclass:nclass + 1], in_=tmp2[:],
                                    axis=mybir.AxisListType.X, op=OP.add)
            nc.vector.tensor_tensor(out=tmp2[:], in0=w2_t[:], in1=ad2[:], op=OP.mult)
            nc.vector.tensor_reduce(out=w2e[:, nclass + 1:d2e], in_=tmp2[:],
                                    axis=mybir.AxisListType.X, op=OP.add)

            ch_c0r = bake["ch_c0r"]
            ch_lag = {}
            for i, b in enumerate(bake["ch_end_blocks"]):
                ch_lag[min(b + 2, nb - 1)] = i
            assert len(ch_lag) == len(bake["ch_end_blocks"]), "chunk lag collision"

            def post_close1(b):
                if b in ch_lag:
                    i = ch_lag[b]
                    r0, r1 = ch_c0r[i], ch_c0r[i + 1]
                    nc.gpsimd.collective_compute(
                        "AllGather", OP.bypass,
                        replica_groups=[list(range(N_CORES))],
                        ins=[shard2[r0:r1, :]],
                        outs=[table2[N_CORES * r0:N_CORES * r1, :]])

            edge_layer(table1, si_t, d1, hh, heads, nhid, ad1own, hs1,
                       close1, post_close1)

            # ---- L2 close: log_softmax -> out
            def close2(b, acc):
                n0 = b * P
                cnt = min(P, npc - n0)
                d1_ = wp.tile([P, 1], F32, tag="d1_")
                nc.vector.tensor_scalar(out=d1_[:], in0=acc[:, nclass:nclass + 1],
                                        scalar1=1e-16, scalar2=None, op0=OP.add)
                r1 = wp.tile([P, 1], F32, tag="r1")
                nc.vector.reciprocal(r1[:], d1_[:])
                z = wp.tile([P, nclass], F32, tag="z")
                nc.vector.tensor_scalar(out=z[:], in0=acc[:, 0:nclass],
                                        scalar1=r1[:, 0:1], scalar2=None,
                                        op0=OP.mult)
                nc.vector.tensor_tensor(out=z[:], in0=z[:], in1=b2_t[:], op=OP.add)
                m = wp.tile([P, 1], F32, tag="m")
                nc.vector.tensor_reduce(out=m[:], in_=z[:],
                                        axis=mybir.AxisListType.X, op=OP.max)
                nc.vector.tensor_scalar(out=z[:], in0=z[:], scalar1=m[:, 0:1],
                                        scalar2=None, op0=OP.subtract)
                e = wp.tile([P, nclass], F32, tag="e")
                se = wp.tile([P, 1], F32, tag="se")
                nc.scalar.activation(e[:], z[:], AF.Exp, accum_out=se[:])
                lse = wp.tile([P, 1], F32, tag="lse")
                nc.scalar.activation(lse[:], se[:], AF.Ln)
                ob = wp.tile([P, nclass], F32, tag="ob")
                nc.vector.tensor_scalar(out=ob[:], in0=z[:], scalar1=lse[:, 0:1],
                                        scalar2=None, op0=OP.subtract)
                nc.sync.dma_start(out[n0:n0 + cnt, :], ob[:cnt, :])

            edge_layer(table2, si2_t, d2, nclass, 1, nclass, ad2own, hs2,
                       close2)

    return nc


_CACHE = {}


def _get_program(bake, nfeat, nhid, heads, nclass):
    key = (bake["nt"], tuple(bake["tiles_per_block"]), nfeat, nhid, heads,
           nclass, G_BUFS, SCRATCH, tuple(sorted(TUNE.items())))
    if key not in _CACHE:
        nc = _build_program(bake, nfeat, nhid, heads, nclass)
        _split_multi_waits(nc)
        _CACHE[key] = nc
    return _CACHE[key]


def build_in_maps(x, edge_index, W1, att_src1, att_dst1, b1, W2, att_src2,
                  att_dst2, b2):
    n_nodes = x.shape[0]
    npc = n_nodes // N_CORES
    bake, src_idx_all, src_idx2_all, dst_loc_all = _host_prep(
        np.asarray(edge_index), n_nodes)
    x = np.asarray(x, np.float32)
    in_maps = []
    for c in range(N_CORES):
        in_maps.append({
            "xT": np.ascontiguousarray(x[c * npc:(c + 1) * npc].T),
            "W1": np.asarray(W1, np.float32),
            "W2": np.asarray(W2, np.float32),
            "asrc1": np.tile(np.asarray(att_src1, np.float32).reshape(1, -1), (P, 1)),
            "adst1": np.tile(np.asarray(att_dst1, np.float32).reshape(1, -1), (P, 1)),
            "asrc2": np.tile(np.asarray(att_src2, np.float32).reshape(1, -1), (P, 1)),
            "adst2": np.tile(np.asarray(att_dst2, np.float32).reshape(1, -1), (P, 1)),
            "b1r": np.tile(np.asarray(b1, np.float32).reshape(1, -1), (P, 1)),
            "b2r": np.tile(np.asarray(b2, np.float32).reshape(1, -1), (P, 1)),
            "srcidx": src_idx_all[c],
            "srcidx2": src_idx2_all[c],
            "iota128": np.tile(np.arange(P, dtype=np.float32), (P, 1)),
            "ident128": np.eye(P, dtype=np.float32),
            "dstloc": dst_loc_all[c],
        })
    return bake, in_maps


def kernel(x, edge_index, W1, att_src1, att_dst1, b1, W2, att_src2, att_dst2, b2):
    from concourse.bass_utils import run_bass_kernel_spmd
    nfeat = x.shape[1]
    heads, nhid = att_src1.shape[1], att_src1.shape[2]
    nclass = att_src2.shape[2]
    bake, in_maps = build_in_maps(x, edge_index, W1, att_src1, att_dst1, b1,
                                  W2, att_src2, att_dst2, b2)
    nc = _get_program(bake, nfeat, nhid, heads, nclass)
    res = run_bass_kernel_spmd(nc, in_maps, core_ids=list(range(N_CORES)))
    return np.concatenate([res.results[c]["out"] for c in range(N_CORES)], axis=0)
